# revision 10
# baseline (speedup 1.0000x reference)
"""AttentionCell (additive attention + GRUCell) Trainium2 kernel.

Self-contained: shards batch (B=256) across 8 NeuronCores (32 each),
runs a Bass/Tile kernel per core, gathers full outputs.

Shapes (hardcoded): T=256, B=256, C=512, H=256, E=128.
Returns (cur_hidden [B,H] f32, alpha [T,B] f32).

Per-core dataflow (bf16 data path, fp32 accumulation):
  feats shard is pre-transposed and pre-cast to bf16 on host, laid out
  [cc, p, bp, bi, t] so C sits on SBUF partitions; HBM traffic is
  8.4 MB/core.  For each bp (pair of batches): feats DMA -> PE matmul
  with W_i2h^T (bf16, fp32 psum) -> ACT tanh(+hidden_proj bias, bf16
  out) -> PE score matmul with block-diagonal w_score placing the two
  batches' score rows on partitions 0 and 32 -> softmax (Exp with
  accumulated sum; scores bounded by ||w_score||_1 so no max
  subtraction) -> normalized alpha rows (bf16) written to DRAM.
  Context: per 4-bp group, alpha rows are broadcast-loaded across 128
  partitions and context^T[c,b] = sum_t alpha*feats is computed as a
  batched DVE bf16 multiply + strided reduce (fp32 out).
  Tail: GRU gates on PE (fp32), pointwise on DVE/ACT.
"""

import numpy as np

import concourse.bacc as bacc
import concourse.mybir as mybir
import concourse.tile as tile
from concourse.bass_utils import run_bass_kernel_spmd

T, B, C, H, E = 256, 256, 512, 256, 128
NCORES = 8
BSH = B // NCORES          # 32 batches per core
NBP = BSH // 2             # 16 batch-pairs per core
NCC = C // 128             # 4 c-chunks
NHC = H // 128             # 2 h-chunks
NKI = (C + E) // 128       # 5 k-chunks for GRU input matmul
NKH = H // 128             # 2 k-chunks for GRU hidden matmul
GRP = 4                    # bp's per context group (8 batches)
NGRP = NBP // GRP

F32 = mybir.dt.float32
BF16 = mybir.dt.bfloat16
AF = mybir.ActivationFunctionType

_COMPILED = None


def _build():
    nc = bacc.Bacc("TRN2", target_bir_lowering=False, debug=False,
                   num_devices=NCORES)

    featsT = nc.dram_tensor("featsT", [NCC, 128, NBP, 2, T], BF16,
                            kind="ExternalInput").ap()
    wi2ht = nc.dram_tensor("wi2ht", [NCC, 128, H], BF16,
                           kind="ExternalInput").ap()
    wsbd = nc.dram_tensor("wsbd", [128, NHC, 2, 64], BF16,
                          kind="ExternalInput").ap()

    def din(name, shape):
        return nc.dram_tensor(name, shape, F32, kind="ExternalInput").ap()

    wh2ht = din("wh2ht", [NHC, 128, H])
    wiht = din("wiht", [NKI, 128, 3 * H])
    whht = din("whht", [NKH, 128, 3 * H])
    pht = din("pht", [NKH, 128, BSH])
    ph = din("ph", [BSH, H])
    embt = din("embt", [E, BSH])
    bh2h = din("bh2h", [NHC, 128, 1])
    bih = din("bih", [1, 3 * H])
    bhh = din("bhh", [1, 3 * H])

    h_out = nc.dram_tensor("h_out", [BSH, H], F32, kind="ExternalOutput").ap()
    alpha_out = nc.dram_tensor("alpha_out", [BSH, T], F32,
                               kind="ExternalOutput").ap()

    with tile.TileContext(nc) as tc:
        with tc.tile_pool(name="w", bufs=1) as wp, \
             tc.tile_pool(name="tanh", bufs=4) as tanhp, \
             tc.tile_pool(name="exp", bufs=2) as expp, \
             tc.tile_pool(name="abc", bufs=2) as abcp, \
             tc.tile_pool(name="prod", bufs=2) as prodp, \
             tc.tile_pool(name="sm", bufs=4) as smp, \
             tc.tile_pool(name="psf", bufs=2, space="PSUM") as psf, \
             tc.tile_pool(name="pse", bufs=2, space="PSUM") as pse, \
             tc.tile_pool(name="psg", bufs=1, space="PSUM") as psg:

            # ---- persistent loads ----
            wi2ht_sb = wp.tile([128, NCC, H], BF16)
            nc.sync.dma_start(wi2ht_sb[:], wi2ht.rearrange("c p h -> p c h"))
            wsbd_sb = wp.tile([128, NHC, 2, 64], BF16)
            nc.sync.dma_start(wsbd_sb[:], wsbd[:])
            wh2ht_sb = wp.tile([128, NHC, H], F32)
            nc.sync.dma_start(wh2ht_sb[:], wh2ht.rearrange("k p h -> p k h"))
            wiht_sb = wp.tile([128, NKI, 3 * H], F32)
            nc.sync.dma_start(wiht_sb[:], wiht.rearrange("k p h -> p k h"))
            whht_sb = wp.tile([128, NKH, 3 * H], F32)
            nc.sync.dma_start(whht_sb[:], whht.rearrange("k p h -> p k h"))
            pht_sb = wp.tile([128, NKH, BSH], F32)
            nc.sync.dma_start(pht_sb[:], pht.rearrange("k p b -> p k b"))
            ph_sb = wp.tile([BSH, H], F32)
            nc.sync.dma_start(ph_sb[:], ph[:])
            embt_sb = wp.tile([E, BSH], F32)
            nc.sync.dma_start(embt_sb[:], embt[:])
            bh2h_sb = wp.tile([128, NHC, 1], F32)
            nc.sync.dma_start(bh2h_sb[:], bh2h.rearrange("k p o -> p k o"))
            bih_sb = wp.tile([1, 3 * H], F32)
            nc.sync.dma_start(bih_sb[:], bih[:])
            bhh_sb = wp.tile([1, 3 * H], F32)
            nc.sync.dma_start(bhh_sb[:], bhh[:])

            # ---- hidden_proj^T [h, b] = W_h2h @ prev_hidden^T + b_h2h ----
            hpt_sb = wp.tile([128, NHC, BSH], F32)
            for hc in range(NHC):
                hp_ps = psf.tile([128, 512], F32, tag="fp")
                for kc in range(NKH):
                    nc.tensor.matmul(
                        hp_ps[:, :BSH],
                        lhsT=wh2ht_sb[:, kc, hc * 128:(hc + 1) * 128],
                        rhs=pht_sb[:, kc, :],
                        start=(kc == 0), stop=(kc == NKH - 1),
                    )
                nc.vector.tensor_scalar_add(
                    hpt_sb[:, hc, :], hp_ps[:, :BSH], bh2h_sb[:, hc, :])

            feats_sb = wp.tile([128, NBP, NCC, 2, T], BF16)
            ctxt_sb = wp.tile([128, NCC, BSH], F32)

            # ---- main pipeline over batch pairs ----
            for bp in range(NBP):
                nc.sync.dma_start(
                    feats_sb[:, bp],
                    featsT[:, :, bp].rearrange("c p b t -> p c b t"))

                em_ps = pse.tile([64, T], F32, tag="em")
                for hc in range(NHC):
                    fp_ps = psf.tile([128, 512], F32, tag="fp")
                    for cc in range(NCC):
                        nc.tensor.matmul(
                            fp_ps[:],
                            lhsT=wi2ht_sb[:, cc, hc * 128:(hc + 1) * 128],
                            rhs=feats_sb[:, bp, cc],
                            start=(cc == 0), stop=(cc == NCC - 1),
                        )
                    for bi in range(2):
                        b = bp * 2 + bi
                        tanh_t = tanhp.tile([128, T], BF16, tag="tanh")
                        nc.scalar.activation(
                            tanh_t[:], fp_ps[:, bi * T:(bi + 1) * T],
                            AF.Tanh, bias=hpt_sb[:, hc, b:b + 1])
                        nc.tensor.matmul(
                            em_ps[:],
                            lhsT=wsbd_sb[:, hc, bi, :],
                            rhs=tanh_t[:],
                            start=(hc == 0 and bi == 0),
                            stop=(hc == NHC - 1 and bi == 1),
                        )

                # softmax rows live at partitions 0 (even b) and 32 (odd b)
                exp_t = expp.tile([64, T], F32, tag="exp")
                sum_t = smp.tile([64, 1], F32, tag="sum")
                nc.scalar.activation(exp_t[:], em_ps[:], AF.Exp,
                                     accum_out=sum_t[:])
                rec_t = smp.tile([64, 1], F32, tag="rec")
                nc.vector.reciprocal(rec_t[:], sum_t[:])
                alpha_t = expp.tile([64, T], F32, tag="alpha")
                nc.scalar.activation(alpha_t[:], exp_t[:], AF.Copy,
                                     scale=rec_t[:])
                for bi in range(2):
                    b = bp * 2 + bi
                    row = slice(32 * bi, 32 * bi + 1)
                    nc.sync.dma_start(alpha_out[b:b + 1, :], alpha_t[row, :])

                # context for the finished group of GRP bp's (8 batches)
                if bp % GRP == GRP - 1:
                    g0 = bp - (GRP - 1)          # first bp of group
                    b0 = g0 * 2                  # first batch of group
                    abc_t = abcp.tile([128, 2 * GRP, T], BF16, tag="abc")
                    nc.gpsimd.dma_start(
                        abc_t[:],
                        alpha_out[b0:b0 + 2 * GRP, :].partition_broadcast(128))
                    for cc in range(NCC):
                        prod_t = prodp.tile([128, GRP, 2, T], BF16, tag="prod")
                        nc.vector.tensor_tensor(
                            out=prod_t[:],
                            in0=feats_sb[:, g0:g0 + GRP, cc],
                            in1=abc_t[:].rearrange(
                                "p (g b) t -> p g b t", g=GRP),
                            op=mybir.AluOpType.mult)
                        nc.vector.tensor_reduce(
                            out=ctxt_sb[:, cc, b0:b0 + 2 * GRP],
                            in_=prod_t[:].rearrange("p g b t -> p (g b) t"),
                            axis=mybir.AxisListType.X,
                            op=mybir.AluOpType.add)

            # ---- GRU tail ----
            # rz accumulates gi_rz + gh_rz (+ both biases) in one psum
            # region; i_n and h_n stay separate for r*h_n.
            rz_tile = psg.tile([BSH, 512], F32, tag="rz")
            gin_tile = psg.tile([BSH, H], F32, tag="gin")
            ghn_tile = psg.tile([BSH, H], F32, tag="ghn")
            rz_ps = rz_tile[:]
            gin_ps = gin_tile[:]
            ghn_ps = ghn_tile[:]
            ones1 = wp.tile([1, BSH], F32)
            nc.vector.memset(ones1[:], 1.0)

            def xT(kc):
                return ctxt_sb[:, kc, :] if kc < NCC else embt_sb[:]

            for kc in range(NKI):
                nc.tensor.matmul(rz_ps, lhsT=xT(kc), rhs=wiht_sb[:, kc, 0:512],
                                 start=(kc == 0), stop=False)
            nc.tensor.matmul(rz_ps, lhsT=ones1[:], rhs=bih_sb[:, 0:512],
                             start=False, stop=False)
            for kc in range(NKH):
                nc.tensor.matmul(rz_ps, lhsT=pht_sb[:, kc, :],
                                 rhs=whht_sb[:, kc, 0:512],
                                 start=False, stop=False)
            nc.tensor.matmul(rz_ps, lhsT=ones1[:], rhs=bhh_sb[:, 0:512],
                             start=False, stop=True)

            for kc in range(NKI):
                nc.tensor.matmul(gin_ps, lhsT=xT(kc),
                                 rhs=wiht_sb[:, kc, 512:768],
                                 start=(kc == 0), stop=False)
            nc.tensor.matmul(gin_ps, lhsT=ones1[:], rhs=bih_sb[:, 512:768],
                             start=False, stop=True)
            for kc in range(NKH):
                nc.tensor.matmul(ghn_ps, lhsT=pht_sb[:, kc, :],
                                 rhs=whht_sb[:, kc, 512:768],
                                 start=(kc == 0), stop=False)
            nc.tensor.matmul(ghn_ps, lhsT=ones1[:], rhs=bhh_sb[:, 512:768],
                             start=False, stop=True)

            rz_sb = wp.tile([BSH, 512], F32)
            nc.scalar.activation(rz_sb[:], rz_ps, AF.Sigmoid)
            rhn_sb = wp.tile([BSH, H], F32)
            nc.vector.tensor_mul(rhn_sb[:], rz_sb[:, 0:H], ghn_ps)
            npre_sb = wp.tile([BSH, H], F32)
            nc.vector.tensor_add(npre_sb[:], gin_ps, rhn_sb[:])
            n_sb = wp.tile([BSH, H], F32)
            nc.scalar.activation(n_sb[:], npre_sb[:], AF.Tanh)
            d_sb = wp.tile([BSH, H], F32)
            nc.vector.tensor_sub(d_sb[:], ph_sb[:], n_sb[:])
            zd_sb = wp.tile([BSH, H], F32)
            nc.vector.tensor_mul(zd_sb[:], rz_sb[:, H:2 * H], d_sb[:])
            hout_sb = wp.tile([BSH, H], F32)
            nc.vector.tensor_add(hout_sb[:], n_sb[:], zd_sb[:])

            nc.sync.dma_start(h_out[:], hout_sb[:])

    nc.compile()
    return nc


def _shard_inputs(inputs):
    import ml_dtypes
    bf = ml_dtypes.bfloat16

    feats = np.asarray(inputs["feats"], dtype=np.float32)
    prev_hidden = np.asarray(inputs["prev_hidden"], dtype=np.float32)
    cur_embeddings = np.asarray(inputs["cur_embeddings"], dtype=np.float32)
    W_i2h = np.asarray(inputs["W_i2h"], dtype=np.float32)
    W_h2h = np.asarray(inputs["W_h2h"], dtype=np.float32)
    b_h2h = np.asarray(inputs["b_h2h"], dtype=np.float32)
    w_score = np.asarray(inputs["w_score"], dtype=np.float32)
    W_ih = np.asarray(inputs["W_ih"], dtype=np.float32)
    W_hh = np.asarray(inputs["W_hh"], dtype=np.float32)
    b_ih = np.asarray(inputs["b_ih"], dtype=np.float32)
    b_hh = np.asarray(inputs["b_hh"], dtype=np.float32)

    wi2ht = np.ascontiguousarray(W_i2h.T.reshape(NCC, 128, H)).astype(bf)
    wh2ht = np.ascontiguousarray(W_h2h.T.reshape(NHC, 128, H))
    wiht = np.ascontiguousarray(W_ih.T.reshape(NKI, 128, 3 * H))
    whht = np.ascontiguousarray(W_hh.T.reshape(NKH, 128, 3 * H))
    bh2h = np.ascontiguousarray(b_h2h.reshape(NHC, 128, 1))
    bih = b_ih.reshape(1, 3 * H)
    bhh = b_hh.reshape(1, 3 * H)

    # block-diag w_score: lhsT for (hc, bi) has w_score chunk hc in col 32*bi
    wsbd = np.zeros((128, NHC, 2, 64), np.float32)
    for hc in range(NHC):
        for bi in range(2):
            wsbd[:, hc, bi, 32 * bi] = w_score[hc * 128:(hc + 1) * 128]
    wsbd = wsbd.astype(bf)

    shared = dict(wi2ht=wi2ht, wh2ht=wh2ht, wiht=wiht, whht=whht,
                  bh2h=bh2h, bih=bih, bhh=bhh, wsbd=wsbd)

    in_maps = []
    for i in range(NCORES):
        b0 = i * BSH
        fsh = feats[:, b0:b0 + BSH, :]                      # [T, 32, C]
        ft = np.ascontiguousarray(
            fsh.transpose(2, 1, 0)                          # [C, 32, T]
            .reshape(NCC, 128, NBP, 2, T)).astype(bf)
        phs = prev_hidden[b0:b0 + BSH, :]                   # [32, H]
        m = dict(shared)
        m["featsT"] = ft
        m["pht"] = np.ascontiguousarray(phs.T.reshape(NKH, 128, BSH))
        m["ph"] = np.ascontiguousarray(phs)
        m["embt"] = np.ascontiguousarray(cur_embeddings[b0:b0 + BSH, :].T)
        in_maps.append(m)
    return in_maps


def kernel(**inputs):
    global _COMPILED
    if _COMPILED is None:
        _COMPILED = _build()
    in_maps = _shard_inputs(inputs)
    res = run_bass_kernel_spmd(_COMPILED, in_maps, core_ids=list(range(NCORES)))
    cur_hidden = np.concatenate(
        [res.results[i]["h_out"] for i in range(NCORES)], axis=0)
    alpha = np.concatenate(
        [res.results[i]["alpha_out"].T for i in range(NCORES)], axis=1)
    return cur_hidden, np.ascontiguousarray(alpha)
